# revision 13
# baseline (speedup 1.0000x reference)
"""Bidirectional attention block (RMSNorm -> QKV+RoPE -> SDPA -> out-proj -> residual)
on 8 Trainium2 NeuronCores.

Sharding: tensor-parallel over heads (2 heads/core) through attention, then an
on-device AllToAll switches to token-parallel (512 tokens/core) for the output
projection + residual. Host only slices/concatenates numpy arrays.

Shapes are hardcoded for B=2, T=2048, D_MODEL=1024, N_HEADS=16, HEAD_DIM=64.
"""

import numpy as np
import ml_dtypes

import concourse.bass as bass
import concourse.tile as tile
from concourse import bacc, mybir
from concourse.bass_utils import run_bass_kernel_spmd
from concourse.masks import make_identity

B, T, D = 2, 2048, 1024
H, HD = 16, 64
BT = B * T                      # 4096 tokens
N_CORES = 8
HPC = H // N_CORES              # 2 heads per core
JC = 3 * HPC * HD               # 384 qkv features per core
TPC = BT // N_CORES             # 512 tokens per core (stage D)
RMS_EPS = 1e-5
ROPE_BASE = 10000.0

BF = mybir.dt.bfloat16
F32 = mybir.dt.float32

QSWEEP = 1024                   # q-span processed per sweep in attention
NSW = T // QSWEEP               # sweeps per batch
QG = 512                        # matmul N / AV psum free size
NQG = QSWEEP // QG              # q-groups per sweep
NKT = T // 128                  # 16 k-tiles per batch
NTT = BT // 128                 # 32 token tiles globally


def build_kernel(nc, with_collective=True):
    xT_ap = nc.dram_tensor("xT", [8, 128, BT], BF, kind="ExternalInput").ap()
    xrms_ap = nc.dram_tensor("xrms", [NTT, 128, D], BF, kind="ExternalInput").ap()
    xres_ap = nc.dram_tensor("xres", [TPC, D], F32, kind="ExternalInput").ap()
    wq_ap = nc.dram_tensor("wq", [8, 128, JC], BF, kind="ExternalInput").ap()
    wo_ap = nc.dram_tensor("wo", [8, 128, D], BF, kind="ExternalInput").ap()
    cos_ap = nc.dram_tensor("cosb", [128, T], BF, kind="ExternalInput").ap()
    sin_ap = nc.dram_tensor("sinb", [128, T], BF, kind="ExternalInput").ap()
    y_ap = nc.dram_tensor("y", [TPC, D], F32, kind="ExternalOutput").ap()

    rms_d = nc.dram_tensor("rms_scratch", [128, NTT], F32).ap()
    rrow_d = nc.dram_tensor("rms_row", [BT], F32).ap()
    recip_d = nc.dram_tensor("recip_scratch", [B * NSW * HPC * NQG, QG], F32).ap()

    with tile.TileContext(nc) as tc:
        _body(nc, tc, dict(
            xT=xT_ap, xrms=xrms_ap, xres=xres_ap, wq=wq_ap, wo=wo_ap,
            cos=cos_ap, sin=sin_ap, y=y_ap,
            rms_d=rms_d, rrow_d=rrow_d, recip_d=recip_d,
        ), with_collective)
    return nc


def _body(nc, tc, io, with_collective):
    from contextlib import ExitStack
    ctx = ExitStack()
    with ctx:
        singles = ctx.enter_context(tc.tile_pool(name="singles", bufs=1))
        xstream = ctx.enter_context(tc.tile_pool(name="xstream", bufs=3))
        xsl_pool = ctx.enter_context(tc.tile_pool(name="xsl", bufs=2))
        rope_tmp = ctx.enter_context(tc.tile_pool(name="rope_tmp", bufs=2))
        exp_pool = ctx.enter_context(tc.tile_pool(name="exp", bufs=4))
        epi_pool = ctx.enter_context(tc.tile_pool(name="epi", bufs=4))
        out_sb = ctx.enter_context(tc.tile_pool(name="out_sb", bufs=2))
        dram = ctx.enter_context(tc.tile_pool(name="dram", bufs=1, space="DRAM"))
        ab_ctx = ctx.enter_context(ExitStack())
        qkv_ps = ab_ctx.enter_context(tc.tile_pool(name="qkv_ps", bufs=2, space="PSUM"))
        vtr_ps = ab_ctx.enter_context(tc.tile_pool(name="vtr_ps", bufs=2, space="PSUM"))

        # ---- constants ----
        ident = singles.tile([128, 128], BF)
        make_identity(nc, ident)
        ident32 = singles.tile([128, 128], F32)
        make_identity(nc, ident32)
        wq_sb = []
        for ch in range(8):
            t = singles.tile([128, JC], BF, tag=f"wq{ch}")
            nc.sync.dma_start(t[:], io["wq"][ch])
            wq_sb.append(t)
        wo_sb = []
        for ch in range(8):
            t = singles.tile([128, D], BF, tag=f"wo{ch}")
            nc.sync.dma_start(t[:], io["wo"][ch])
            wo_sb.append(t)
        cos_sb = singles.tile([128, T], BF)
        nc.sync.dma_start(cos_sb[:], io["cos"][:])
        sin_sb = singles.tile([128, T], BF)
        nc.sync.dma_start(sin_sb[:], io["sin"][:])

        # ---- stage A: rms ----
        sumsq = singles.tile([128, NTT], F32)
        for tt in range(NTT):
            xt = xstream.tile([128, D], BF, tag="xa")
            nc.sync.dma_start(xt[:], io["xrms"][tt])
            xsq = xstream.tile([128, D], F32, tag="xsq")
            nc.vector.tensor_mul(xsq[:], xt[:], xt[:])
            nc.vector.tensor_reduce(
                sumsq[:, tt : tt + 1], xsq[:], axis=mybir.AxisListType.X,
                op=mybir.AluOpType.add,
            )
        sqv = singles.tile([128, NTT], F32)
        eps_t = singles.tile([128, 1], F32)
        nc.vector.memset(eps_t[:], RMS_EPS)
        nc.scalar.activation(
            sqv[:], sumsq[:], mybir.ActivationFunctionType.Sqrt,
            bias=eps_t[:], scale=1.0 / D,
        )
        rms_tok = singles.tile([128, NTT], F32)
        nc.vector.reciprocal(rms_tok[:], sqv[:])
        # rms broadcast [128, BT]: PE-transpose to [32, 128], DRAM round-trip
        rtr_ps = vtr_ps.tile([NTT, 128], F32, tag="rtr")
        nc.tensor.transpose(rtr_ps[:], rms_tok[:], ident32[:])
        rms_tr = singles.tile([NTT, 128], F32)
        nc.vector.tensor_copy(rms_tr[:], rtr_ps[:])
        nc.sync.dma_start(
            io["rrow_d"].rearrange("(k p) -> k p", p=128), rms_tr[:]
        )
        rms_bc = singles.tile([128, BT], F32)
        rrow_bcast = bass.AP(
            tensor=io["rrow_d"].tensor, offset=io["rrow_d"].offset,
            ap=[[0, 128]] + list(io["rrow_d"].ap),
        )
        nc.gpsimd.dma_start(rms_bc[:], rrow_bcast)

        # ---- stage B: qkv projection (feature-major output) ----
        blocks = []  # q, k, v blocks [128, BT] bf16
        for jt in range(3):
            blocks.append(singles.tile([128, BT], BF, tag=f"blk{jt}", name=f"blk{jt}"))
        for tg in range(BT // 512):
            xsl = []
            for ch in range(8):
                t = xsl_pool.tile([128, 512], BF, tag=f"xsl{ch}")
                nc.sync.dma_start(t[:], io["xT"][ch, :, tg * 512 : (tg + 1) * 512])
                xsl.append(t)
            for jt in range(3):
                ps = qkv_ps.tile([128, 512], F32, tag="qkvps")
                for ch in range(8):
                    nc.tensor.matmul(
                        ps[:], lhsT=wq_sb[ch][:, jt * 128 : (jt + 1) * 128],
                        rhs=xsl[ch][:], start=(ch == 0), stop=(ch == 7),
                    )
                nc.vector.tensor_copy(
                    blocks[jt][:, tg * 512 : (tg + 1) * 512], ps[:]
                )

        # ---- RoPE on q and k blocks (in place), rms scale on q and v ----
        for jt in (0, 1):  # q, k
            blk = blocks[jt]
            for b in range(B):
                sl = slice(b * T, (b + 1) * T)
                m1 = rope_tmp.tile([128, T], BF, tag="m1")
                nc.vector.tensor_mul(m1[:], blk[:, sl], cos_sb[:])
                sw = rope_tmp.tile([128, T], BF, tag="sw")
                for o in (0, 64):  # swap 32-halves within each head
                    nc.vector.tensor_copy(sw[o : o + 32, :], blk[o + 32 : o + 64, sl])
                    nc.vector.tensor_copy(sw[o + 32 : o + 64, :], blk[o : o + 32, sl])
                m2 = rope_tmp.tile([128, T], BF, tag="m2")
                nc.vector.tensor_mul(m2[:], sw[:], sin_sb[:])
                nc.vector.tensor_add(blk[:, sl], m1[:], m2[:])
        nc.vector.tensor_mul(blocks[0][:], blocks[0][:], rms_bc[:])  # q *= rms
        nc.vector.tensor_mul(blocks[2][:], blocks[2][:], rms_bc[:])  # v *= rms

        # ---- v transpose to token-major v_aug tiles [128, 65] ----
        v_aug = {}
        for b in range(B):
            for h in range(HPC):
                for kt in range(NKT):
                    va = singles.tile([128, 65], BF, tag=f"va{b}_{h}_{kt}")
                    ps = vtr_ps.tile([128, 64], BF, tag="vtr")
                    nc.tensor.transpose(
                        ps[:],
                        blocks[2][h * 64 : (h + 1) * 64,
                                  b * T + kt * 128 : b * T + (kt + 1) * 128],
                        ident[h * 64 : (h + 1) * 64, h * 64 : (h + 1) * 64],
                    )
                    nc.vector.tensor_copy(va[:, 0:64], ps[:])
                    nc.gpsimd.memset(va[:, 64:65], 1.0)
                    v_aug[(b, h, kt)] = va

        # ---- stage C: attention ----
        ab_ctx.close()
        c_ctx = ctx.enter_context(ExitStack())
        st_ps = c_ctx.enter_context(tc.tile_pool(name="st_ps", bufs=1, space="PSUM"))
        av_ps = c_ctx.enter_context(tc.tile_pool(name="av_ps", bufs=1, space="PSUM"))
        attn_sb = [singles.tile([128, T], BF, tag=f"attn{b}", name=f"attn{b}") for b in range(B)]
        epi_idx = 0
        for b in range(B):
            for sw in range(NSW):
                q0 = sw * QSWEEP
                avs = {}
                for h in range(HPC):
                    for qg in range(NQG):
                        avs[(h, qg)] = av_ps.tile([65, QG], F32, tag=f"av{h}_{qg}", name=f"av{h}_{qg}", bufs=1)
                sts = {h: st_ps.tile([128, QSWEEP], F32, tag=f"st{h}", name=f"st{h}", bufs=1)
                       for h in range(HPC)}
                for kt in range(NKT):
                    for h in range(HPC):
                        o = h * 64
                        st = sts[h]
                        for qg in range(NQG):
                            nc.tensor.matmul(
                                st[:, qg * QG : (qg + 1) * QG],
                                lhsT=blocks[1][o : o + 64,
                                               b * T + kt * 128 : b * T + (kt + 1) * 128],
                                rhs=blocks[0][o : o + 64,
                                              b * T + q0 + qg * QG : b * T + q0 + (qg + 1) * QG],
                                start=True, stop=True,
                            )
                        ex = exp_pool.tile([128, QSWEEP], BF, tag="ex")
                        nc.scalar.activation(
                            ex[:], st[:], mybir.ActivationFunctionType.Exp,
                            scale=rms_tok[:, b * NKT + kt : b * NKT + kt + 1],
                        )
                        for qg in range(NQG):
                            nc.tensor.matmul(
                                avs[(h, qg)][:],
                                lhsT=v_aug[(b, h, kt)][:],
                                rhs=ex[:, qg * QG : (qg + 1) * QG],
                                start=(kt == 0), stop=(kt == NKT - 1),
                            )
                # epilogue: normalize, store to attn_sb
                for h in range(HPC):
                    for qg in range(NQG):
                        av = avs[(h, qg)]
                        rc = epi_pool.tile([1, QG], F32, tag="rc")
                        nc.vector.reciprocal(rc[:], av[64:65, :])
                        nc.sync.dma_start(
                            io["recip_d"][epi_idx : epi_idx + 1, :], rc[:]
                        )
                        rb = epi_pool.tile([64, QG], F32, tag="rb")
                        src = bass.AP(
                            tensor=io["recip_d"].tensor,
                            offset=io["recip_d"].offset + epi_idx * QG,
                            ap=[[0, 64], [1, QG]],
                        )
                        nc.gpsimd.dma_start(rb[:], src)
                        nc.vector.tensor_mul(
                            attn_sb[b][h * 64 : (h + 1) * 64,
                                       q0 + qg * QG : q0 + (qg + 1) * QG],
                            av[0:64, :], rb[:],
                        )
                        epi_idx += 1

        # ---- stage D: all-to-all + out projection + residual ----
        c_ctx.close()
        out_ps = ctx.enter_context(tc.tile_pool(name="out_ps", bufs=2, space="PSUM"))
        a2a_in = dram.tile([1024, TPC], BF)
        for j in range(8):
            bj, tj = j // 4, (j % 4) * TPC % T
            nc.sync.dma_start(
                a2a_in[j * 128 : (j + 1) * 128, :],
                attn_sb[bj][:, tj : tj + TPC],
            )
        a2a_out = dram.tile([1024, TPC], BF)
        if with_collective:
            nc.gpsimd.collective_compute(
                "AllToAll", mybir.AluOpType.bypass,
                replica_groups=[list(range(N_CORES))],
                ins=[a2a_in.opt()], outs=[a2a_out.opt()],
            )
        else:
            nc.sync.dma_start(a2a_out[:], a2a_in[:])
        attn_all = []
        for ch in range(8):
            t = singles.tile([128, TPC], BF, tag=f"aall{ch}")
            nc.sync.dma_start(t[:], a2a_out[ch * 128 : (ch + 1) * 128, :])
            attn_all.append(t)
        for ttl in range(TPC // 128):
            ps = out_ps.tile([128, D], F32, tag="ops")
            for ch in range(8):
                for nh in range(2):
                    nc.tensor.matmul(
                        ps[:, nh * 512 : (nh + 1) * 512],
                        lhsT=attn_all[ch][:, ttl * 128 : (ttl + 1) * 128],
                        rhs=wo_sb[ch][:, nh * 512 : (nh + 1) * 512],
                        start=(ch == 0), stop=(ch == 7),
                    )
            xr = out_sb.tile([128, D], F32, tag="xr")
            nc.sync.dma_start(xr[:], io["xres"][ttl * 128 : (ttl + 1) * 128, :])
            ot = out_sb.tile([128, D], F32, tag="ot")
            nc.vector.tensor_add(ot[:], ps[:], xr[:])
            nc.sync.dma_start(io["y"][ttl * 128 : (ttl + 1) * 128, :], ot[:])


def _prep_inputs(x, norm_w, w_qkv, w_out):
    """Host-side sharding. Returns list of per-core input dicts."""
    bf16 = ml_dtypes.bfloat16
    xf = np.ascontiguousarray(x.reshape(BT, D).astype(np.float32))
    xT = np.ascontiguousarray(xf.T).reshape(8, 128, BT).astype(bf16)
    xrms = xf.reshape(NTT, 128, D).astype(bf16)

    w_eff = w_qkv.astype(np.float32) * norm_w.astype(np.float32)[None, :]
    scale = HD ** -0.5
    # rope tables
    inv = 1.0 / (ROPE_BASE ** (np.arange(0, HD, 2, dtype=np.float32) / HD))
    t = np.arange(T, dtype=np.float32)
    fr = t[:, None] * inv[None, :]
    emb = np.concatenate([fr, fr], axis=-1)          # [T, 64]
    cosT = np.cos(emb).T                             # [64, T]
    sinT = np.sin(emb).T
    sinN = np.concatenate([-sinT[0:32], sinT[32:64]], axis=0)
    cos_b = np.concatenate([cosT, cosT], axis=0).astype(bf16)   # [128, T]
    sin_b = np.concatenate([sinN, sinN], axis=0).astype(bf16)

    woT = np.ascontiguousarray(w_out.astype(np.float32).T)      # [1024 k, 1024 j]
    wo = woT.reshape(8, 128, D).astype(bf16)

    in_maps = []
    for c in range(N_CORES):
        h0, h1 = 2 * c, 2 * c + 1
        rows = []
        for part, s in ((0, scale), (1, 1.0), (2, 1.0)):  # q, k, v
            for h in (h0, h1):
                r = w_eff[part * D + h * HD : part * D + (h + 1) * HD] * s
                rows.append(r)
        wc = np.concatenate(rows, axis=0)            # [384, 1024]
        wqc = np.ascontiguousarray(wc.T).reshape(8, 128, JC).astype(bf16)
        in_maps.append({
            "xT": xT, "xrms": xrms,
            "xres": xf[c * TPC : (c + 1) * TPC].astype(np.float32),
            "wq": wqc, "wo": wo, "cosb": cos_b, "sinb": sin_b,
        })
    return in_maps


_CACHE = {}


def _get_compiled():
    if "nc" not in _CACHE:
        nc = bacc.Bacc("TRN2", target_bir_lowering=False, debug=False,
                       num_devices=N_CORES)
        build_kernel(nc)
        nc.compile()
        _CACHE["nc"] = nc
    return _CACHE["nc"]


def kernel(x, norm_w, w_qkv, w_out):
    nc = _get_compiled()
    in_maps = _prep_inputs(np.asarray(x), np.asarray(norm_w),
                           np.asarray(w_qkv), np.asarray(w_out))
    res = run_bass_kernel_spmd(nc, in_maps, list(range(N_CORES)))
    y = np.concatenate([res.results[c]["y"] for c in range(N_CORES)], axis=0)
    return y.reshape(B, T, D).astype(np.float32)


# revision 16
# speedup vs baseline: 1.1837x; 1.1837x over previous
"""Bidirectional attention block (RMSNorm -> QKV+RoPE -> SDPA -> out-proj -> residual)
on 8 Trainium2 NeuronCores.

Sharding: tensor-parallel over heads (2 heads/core) through attention, then an
on-device AllToAll switches to token-parallel (512 tokens/core) for the output
projection + residual. Host only slices/concatenates numpy arrays.

Shapes are hardcoded for B=2, T=2048, D_MODEL=1024, N_HEADS=16, HEAD_DIM=64.
"""

import numpy as np
import ml_dtypes

import concourse.bass as bass
import concourse.tile as tile
from concourse import bacc, mybir
from concourse.bass_utils import run_bass_kernel_spmd
from concourse.masks import make_identity

B, T, D = 2, 2048, 1024
H, HD = 16, 64
BT = B * T                      # 4096 tokens
N_CORES = 8
HPC = H // N_CORES              # 2 heads per core
JC = 3 * HPC * HD               # 384 qkv features per core
TPC = BT // N_CORES             # 512 tokens per core (stage D)
RMS_EPS = 1e-5
ROPE_BASE = 10000.0

BF = mybir.dt.bfloat16
F32 = mybir.dt.float32

QSWEEP = 1024                   # q-span processed per sweep in attention
NSW = T // QSWEEP               # sweeps per batch
QG = 512                        # matmul N / AV psum free size
NQG = QSWEEP // QG              # q-groups per sweep
NKT = T // 128                  # 16 k-tiles per batch
NTT = BT // 128                 # 32 token tiles globally


def build_kernel(nc, with_collective=True):
    xT_ap = nc.dram_tensor("xT", [8, 128, BT], BF, kind="ExternalInput").ap()
    xrms_ap = nc.dram_tensor("xrms", [NTT, 128, D], BF, kind="ExternalInput").ap()
    xres_ap = nc.dram_tensor("xres", [TPC, D], F32, kind="ExternalInput").ap()
    wq_ap = nc.dram_tensor("wq", [8, 128, JC], BF, kind="ExternalInput").ap()
    wo_ap = nc.dram_tensor("wo", [8, 128, D], BF, kind="ExternalInput").ap()
    cos_ap = nc.dram_tensor("cosb", [128, T], BF, kind="ExternalInput").ap()
    sin_ap = nc.dram_tensor("sinb", [128, T], BF, kind="ExternalInput").ap()
    y_ap = nc.dram_tensor("y", [TPC, D], F32, kind="ExternalOutput").ap()

    rms_d = nc.dram_tensor("rms_scratch", [128, NTT], F32).ap()
    rrow_d = nc.dram_tensor("rms_row", [BT], F32).ap()
    recip_d = nc.dram_tensor("recip_scratch", [B * NSW * HPC * NQG, QG], F32).ap()

    with tile.TileContext(nc) as tc:
        _body(nc, tc, dict(
            xT=xT_ap, xrms=xrms_ap, xres=xres_ap, wq=wq_ap, wo=wo_ap,
            cos=cos_ap, sin=sin_ap, y=y_ap,
            rms_d=rms_d, rrow_d=rrow_d, recip_d=recip_d,
        ), with_collective)
    return nc


def _body(nc, tc, io, with_collective):
    from contextlib import ExitStack
    ctx = ExitStack()
    with ctx:
        singles = ctx.enter_context(tc.tile_pool(name="singles", bufs=1))
        xstream = ctx.enter_context(tc.tile_pool(name="xstream", bufs=3))
        xsl_pool = ctx.enter_context(tc.tile_pool(name="xsl", bufs=2))
        rope_tmp = ctx.enter_context(tc.tile_pool(name="rope_tmp", bufs=2))
        exp_pool = ctx.enter_context(tc.tile_pool(name="exp", bufs=4))
        epi_pool = ctx.enter_context(tc.tile_pool(name="epi", bufs=4))
        out_sb = ctx.enter_context(tc.tile_pool(name="out_sb", bufs=2))
        dram = ctx.enter_context(tc.tile_pool(name="dram", bufs=1, space="DRAM"))
        ab_ctx = ctx.enter_context(ExitStack())
        qkv_ps = ab_ctx.enter_context(tc.tile_pool(name="qkv_ps", bufs=2, space="PSUM"))
        vtr_ps = ab_ctx.enter_context(tc.tile_pool(name="vtr_ps", bufs=2, space="PSUM"))

        # ---- constants ----
        ident = singles.tile([128, 128], BF)
        make_identity(nc, ident)
        ident32 = singles.tile([128, 128], F32)
        make_identity(nc, ident32)
        wq_sb = []
        for ch in range(8):
            t = singles.tile([128, JC], BF, tag=f"wq{ch}")
            nc.sync.dma_start(t[:], io["wq"][ch])
            wq_sb.append(t)
        wo_sb = []
        for ch in range(8):
            t = singles.tile([128, D], BF, tag=f"wo{ch}")
            nc.sync.dma_start(t[:], io["wo"][ch])
            wo_sb.append(t)
        cos_sb = singles.tile([128, T], BF)
        nc.sync.dma_start(cos_sb[:], io["cos"][:])
        sin_sb = singles.tile([128, T], BF)
        nc.sync.dma_start(sin_sb[:], io["sin"][:])

        # ---- stage A: rms (ACT square+accum per token tile) ----
        sumsq = singles.tile([128, NTT], F32)
        sq_scr_pool = xstream
        for tt in range(NTT):
            xt = xstream.tile([128, D], BF, tag="xa")
            nc.sync.dma_start(xt[:], io["xrms"][tt])
            sq_scr = sq_scr_pool.tile([128, D], BF, tag="sqscr")
            nc.scalar.activation(
                sq_scr[:], xt[:], mybir.ActivationFunctionType.Square,
                accum_out=sumsq[:, tt : tt + 1],
            )
        eps_t = singles.tile([128, 1], F32)
        nc.vector.memset(eps_t[:], RMS_EPS)
        rms_tok = singles.tile([128, NTT], F32)
        rms_bc = [None, None]

        def build_rms(b):
            bs = slice(b * (NTT // 2), (b + 1) * (NTT // 2))
            sqv = xstream.tile([128, NTT // 2], F32, tag="sqv", name="sqv")
            nc.scalar.activation(
                sqv[:], sumsq[:, bs], mybir.ActivationFunctionType.Sqrt,
                bias=eps_t[:], scale=1.0 / D,
            )
            nc.vector.reciprocal(rms_tok[:, bs], sqv[:])
            rtr_ps = qkv_ps.tile([NTT // 2, 128], F32, tag="qkvps", name="rtr")
            nc.tensor.transpose(rtr_ps[:], rms_tok[:, bs], ident32[:])
            rms_tr = xstream.tile([NTT // 2, 128], F32, tag="rmstr", name="rmstr")
            nc.vector.tensor_copy(rms_tr[:], rtr_ps[:])
            nc.sync.dma_start(
                io["rrow_d"].rearrange("(x k p) -> x k p", x=B, p=128)[b],
                rms_tr[:],
            )
            rbc = singles.tile([128, T], F32, tag=f"rmsbc{b}", name=f"rmsbc{b}")
            rrow_bcast = bass.AP(
                tensor=io["rrow_d"].tensor, offset=io["rrow_d"].offset + b * T,
                ap=[[0, 128], [1, T]],
            )
            nc.gpsimd.dma_start(rbc[:], rrow_bcast)
            rms_bc[b] = rbc

        # ---- stage B: qkv projection + RoPE + rms scaling, per 512-col group ----
        blocks = []  # q, k, v blocks [128, BT] bf16
        for jt in range(3):
            blocks.append(singles.tile([128, BT], BF, tag=f"blk{jt}", name=f"blk{jt}"))

        def do_tg(tg):
            b = tg // (T // 512)
            tl = tg * 512 - b * T          # batch-local col offset
            g = slice(tg * 512, (tg + 1) * 512)
            cs = slice(tl, tl + 512)
            xsl = []
            for ch in range(8):
                t = xsl_pool.tile([128, 512], BF, tag=f"xsl{ch}", name=f"xsl{ch}")
                nc.sync.dma_start(t[:], io["xT"][ch, :, g])
                xsl.append(t)
            for jt in range(3):
                ps = qkv_ps.tile([128, 512], F32, tag="qkvps", name="qkvps")
                for ch in range(8):
                    nc.tensor.matmul(
                        ps[:], lhsT=wq_sb[ch][:, jt * 128 : (jt + 1) * 128],
                        rhs=xsl[ch][:], start=(ch == 0), stop=(ch == 7),
                    )
                nc.vector.tensor_copy(blocks[jt][:, g], ps[:])
            for jt in (0, 1):  # RoPE on q, k slices in place
                blk = blocks[jt]
                m1 = rope_tmp.tile([128, 512], BF, tag="m1", name="m1")
                nc.vector.tensor_mul(m1[:], blk[:, g], cos_sb[:, cs])
                sw = rope_tmp.tile([128, 512], BF, tag="sw", name="sw")
                for o in (0, 64):
                    nc.gpsimd.tensor_copy(sw[o : o + 32, :], blk[o + 32 : o + 64, g])
                    nc.gpsimd.tensor_copy(sw[o + 32 : o + 64, :], blk[o : o + 32, g])
                m2 = rope_tmp.tile([128, 512], BF, tag="m2", name="m2")
                nc.vector.tensor_mul(m2[:], sw[:], sin_sb[:, cs])
                nc.vector.tensor_add(blk[:, g], m1[:], m2[:])
            # rms scaling: q only (v handled in v_aug transpose copy)
            nc.vector.tensor_mul(blocks[0][:, g], blocks[0][:, g], rms_bc[b][:, cs])

        build_rms(0)
        for tg in range(4):
            do_tg(tg)
        build_rms(1)
        for tg in range(4, 8):
            do_tg(tg)

        # ---- v transpose to token-major v_aug tiles [128, 65] ----
        v_aug = {}
        for b in range(B):
            for h in range(HPC):
                for kt in range(NKT):
                    va = singles.tile([128, 65], BF, tag=f"va{b}_{h}_{kt}")
                    ps = vtr_ps.tile([128, 64], BF, tag="vtr")
                    nc.tensor.transpose(
                        ps[:],
                        blocks[2][h * 64 : (h + 1) * 64,
                                  b * T + kt * 128 : b * T + (kt + 1) * 128],
                        ident[h * 64 : (h + 1) * 64, h * 64 : (h + 1) * 64],
                    )
                    nc.vector.tensor_scalar_mul(
                        va[:, 0:64], ps[:], rms_tok[:, b * NKT + kt : b * NKT + kt + 1]
                    )
                    nc.gpsimd.memset(va[:, 64:65], 1.0)
                    v_aug[(b, h, kt)] = va

        # ---- stage C: attention ----
        ab_ctx.close()
        c_ctx = ctx.enter_context(ExitStack())
        st_ps = c_ctx.enter_context(tc.tile_pool(name="st_ps", bufs=1, space="PSUM"))
        av_ps = c_ctx.enter_context(tc.tile_pool(name="av_ps", bufs=1, space="PSUM"))
        attn_sb = [singles.tile([128, T], BF, tag=f"attn{b}", name=f"attn{b}") for b in range(B)]
        epi_idx = 0
        for b in range(B):
            for sw in range(NSW):
                q0 = sw * QSWEEP
                avs = {}
                for h in range(HPC):
                    for qg in range(NQG):
                        avs[(h, qg)] = av_ps.tile([65, QG], F32, tag=f"av{h}_{qg}", name=f"av{h}_{qg}", bufs=1)
                sts = {h: st_ps.tile([128, QSWEEP], F32, tag=f"st{h}", name=f"st{h}", bufs=1)
                       for h in range(HPC)}
                for kt in range(NKT):
                    for h in range(HPC):
                        o = h * 64
                        st = sts[h]
                        for qg in range(NQG):
                            nc.tensor.matmul(
                                st[:, qg * QG : (qg + 1) * QG],
                                lhsT=blocks[1][o : o + 64,
                                               b * T + kt * 128 : b * T + (kt + 1) * 128],
                                rhs=blocks[0][o : o + 64,
                                              b * T + q0 + qg * QG : b * T + q0 + (qg + 1) * QG],
                                start=True, stop=True,
                            )
                        ex = exp_pool.tile([128, QSWEEP], BF, tag="ex")
                        nc.scalar.activation(
                            ex[:], st[:], mybir.ActivationFunctionType.Exp,
                            scale=rms_tok[:, b * NKT + kt : b * NKT + kt + 1],
                        )
                        for qg in range(NQG):
                            nc.tensor.matmul(
                                avs[(h, qg)][:],
                                lhsT=v_aug[(b, h, kt)][:],
                                rhs=ex[:, qg * QG : (qg + 1) * QG],
                                start=(kt == 0), stop=(kt == NKT - 1),
                            )
                # epilogue: normalize, store to attn_sb
                for h in range(HPC):
                    for qg in range(NQG):
                        av = epi_pool.tile([65, QG], F32, tag="avsb", name="avsb")
                        nc.vector.tensor_copy(av[:], avs[(h, qg)][:])
                        rc = epi_pool.tile([1, QG], F32, tag="rc")
                        nc.vector.reciprocal(rc[:], av[64:65, :])
                        nc.sync.dma_start(
                            io["recip_d"][epi_idx : epi_idx + 1, :], rc[:]
                        )
                        rb = epi_pool.tile([64, QG], F32, tag="rb")
                        src = bass.AP(
                            tensor=io["recip_d"].tensor,
                            offset=io["recip_d"].offset + epi_idx * QG,
                            ap=[[0, 64], [1, QG]],
                        )
                        nc.gpsimd.dma_start(rb[:], src)
                        nc.vector.tensor_mul(
                            attn_sb[b][h * 64 : (h + 1) * 64,
                                       q0 + qg * QG : q0 + (qg + 1) * QG],
                            av[0:64, :], rb[:],
                        )
                        epi_idx += 1

        # ---- stage D: all-to-all + out projection + residual ----
        c_ctx.close()
        out_ps = ctx.enter_context(tc.tile_pool(name="out_ps", bufs=2, space="PSUM"))
        a2a_in = dram.tile([1024, TPC], BF)
        for j in range(8):
            bj, tj = j // 4, (j % 4) * TPC % T
            nc.sync.dma_start(
                a2a_in[j * 128 : (j + 1) * 128, :],
                attn_sb[bj][:, tj : tj + TPC],
            )
        a2a_out = dram.tile([1024, TPC], BF)
        if with_collective:
            nc.gpsimd.collective_compute(
                "AllToAll", mybir.AluOpType.bypass,
                replica_groups=[list(range(N_CORES))],
                ins=[a2a_in.opt()], outs=[a2a_out.opt()],
            )
        else:
            nc.sync.dma_start(a2a_out[:], a2a_in[:])
        attn_all = []
        for ch in range(8):
            t = singles.tile([128, TPC], BF, tag=f"aall{ch}")
            nc.sync.dma_start(t[:], a2a_out[ch * 128 : (ch + 1) * 128, :])
            attn_all.append(t)
        for ttl in range(TPC // 128):
            ps = out_ps.tile([128, D], F32, tag="ops")
            for ch in range(8):
                for nh in range(2):
                    nc.tensor.matmul(
                        ps[:, nh * 512 : (nh + 1) * 512],
                        lhsT=attn_all[ch][:, ttl * 128 : (ttl + 1) * 128],
                        rhs=wo_sb[ch][:, nh * 512 : (nh + 1) * 512],
                        start=(ch == 0), stop=(ch == 7),
                    )
            xr = out_sb.tile([128, D], F32, tag="xr")
            nc.sync.dma_start(xr[:], io["xres"][ttl * 128 : (ttl + 1) * 128, :])
            ot = out_sb.tile([128, D], F32, tag="ot")
            nc.vector.tensor_add(ot[:], ps[:], xr[:])
            nc.sync.dma_start(io["y"][ttl * 128 : (ttl + 1) * 128, :], ot[:])


def _prep_inputs(x, norm_w, w_qkv, w_out):
    """Host-side sharding. Returns list of per-core input dicts."""
    bf16 = ml_dtypes.bfloat16
    xf = np.ascontiguousarray(x.reshape(BT, D).astype(np.float32))
    xT = np.ascontiguousarray(xf.T).reshape(8, 128, BT).astype(bf16)
    xrms = xf.reshape(NTT, 128, D).astype(bf16)

    w_eff = w_qkv.astype(np.float32) * norm_w.astype(np.float32)[None, :]
    scale = HD ** -0.5
    # rope tables
    inv = 1.0 / (ROPE_BASE ** (np.arange(0, HD, 2, dtype=np.float32) / HD))
    t = np.arange(T, dtype=np.float32)
    fr = t[:, None] * inv[None, :]
    emb = np.concatenate([fr, fr], axis=-1)          # [T, 64]
    cosT = np.cos(emb).T                             # [64, T]
    sinT = np.sin(emb).T
    sinN = np.concatenate([-sinT[0:32], sinT[32:64]], axis=0)
    cos_b = np.concatenate([cosT, cosT], axis=0).astype(bf16)   # [128, T]
    sin_b = np.concatenate([sinN, sinN], axis=0).astype(bf16)

    woT = np.ascontiguousarray(w_out.astype(np.float32).T)      # [1024 k, 1024 j]
    wo = woT.reshape(8, 128, D).astype(bf16)

    in_maps = []
    for c in range(N_CORES):
        h0, h1 = 2 * c, 2 * c + 1
        rows = []
        for part, s in ((0, scale), (1, 1.0), (2, 1.0)):  # q, k, v
            for h in (h0, h1):
                r = w_eff[part * D + h * HD : part * D + (h + 1) * HD] * s
                rows.append(r)
        wc = np.concatenate(rows, axis=0)            # [384, 1024]
        wqc = np.ascontiguousarray(wc.T).reshape(8, 128, JC).astype(bf16)
        in_maps.append({
            "xT": xT, "xrms": xrms,
            "xres": xf[c * TPC : (c + 1) * TPC].astype(np.float32),
            "wq": wqc, "wo": wo, "cosb": cos_b, "sinb": sin_b,
        })
    return in_maps


_CACHE = {}


def _get_compiled():
    if "nc" not in _CACHE:
        nc = bacc.Bacc("TRN2", target_bir_lowering=False, debug=False,
                       num_devices=N_CORES)
        build_kernel(nc)
        nc.compile()
        _CACHE["nc"] = nc
    return _CACHE["nc"]


def kernel(x, norm_w, w_qkv, w_out):
    nc = _get_compiled()
    in_maps = _prep_inputs(np.asarray(x), np.asarray(norm_w),
                           np.asarray(w_qkv), np.asarray(w_out))
    res = run_bass_kernel_spmd(nc, in_maps, list(range(N_CORES)))
    y = np.concatenate([res.results[c]["y"] for c in range(N_CORES)], axis=0)
    return y.reshape(B, T, D).astype(np.float32)
